# revision 7
# baseline (speedup 1.0000x reference)
"""CRAFT OHEM loss on 8 trn2 NeuronCores — data-parallel over batch.

Math: with uniform-random inputs, n_neg_total (≈0.25·N) is always far below
NEG_RATIO·n_pos (≈2.25·N), so the reference's OHEM top-k selects *all*
negatives and every branch of the loss reduces to masked global sums:

    pos  = (region_target > .5) | (affinity_target > .5)   [= max(rt,at) > .5]
    S_pos_r = Σ pos·(rp-rt)²    S_tot_r = Σ (rp-rt)²       (same for affinity)
    n_pos   = Σ pos             n_neg_tot = N - n_pos

    region_loss   = S_pos_r/n_pos + (S_tot_r - S_pos_r)/n_neg
    affinity_loss = S_pos_a/n_pos + (S_tot_a - S_pos_a)/n_neg

Each core streams its 4·(4,1,640,640)-batch shard through SBUF once
(memory-bound) and emits 5 per-partition partial-sum columns per chunk via
fused accumulating ops (ACT square+accum, DVE scalar_tensor_tensor/
tensor_scalar accum, Pool max).  The host combines partials in float64 and
falls back to an exact numpy OHEM in the (unreachable for this input
distribution) case n_neg_tot > NEG_RATIO·n_pos.
"""

import numpy as np

import concourse.bass as bass
import concourse.bacc as bacc
import concourse.mybir as mybir
from concourse.tile import TileContext
from concourse.bass_utils import run_bass_kernel_spmd

N_CORES = 8
B, H, W = 32, 640, 640
N_TOTAL = B * H * W                  # 13_107_200
PER_CORE = N_TOTAL // N_CORES        # 1_638_400
P = 128
F_TOT = PER_CORE // P                # 12_800
F_CHUNK = 1600                       # -> 8 chunks, 0.82 MB per input DMA
NEG_RATIO = 3.0

_F32 = mybir.dt.float32


def build_nc(f_tot: int = F_TOT, f_chunk: int = F_CHUNK) -> bass.Bass:
    assert f_tot % f_chunk == 0
    nchunk = f_tot // f_chunk

    nc = bacc.Bacc(None)
    rp = nc.dram_tensor("region_pred", [P, f_tot], _F32, kind="ExternalInput")
    ap = nc.dram_tensor("affinity_pred", [P, f_tot], _F32, kind="ExternalInput")
    rt = nc.dram_tensor("region_target", [P, f_tot], _F32, kind="ExternalInput")
    at = nc.dram_tensor("affinity_target", [P, f_tot], _F32, kind="ExternalInput")
    # per-chunk partial-sum columns; cols [0:nchunk] = region, [nchunk:] = affinity
    sd_out = nc.dram_tensor("stats_dve", [P, 2 * nchunk], _F32, kind="ExternalOutput")
    sa_out = nc.dram_tensor("stats_act", [P, 2 * nchunk], _F32, kind="ExternalOutput")
    sp_out = nc.dram_tensor("stats_pool", [P, nchunk], _F32, kind="ExternalOutput")

    SQ = mybir.ActivationFunctionType.Square
    IS_GT = mybir.AluOpType.is_gt
    MULT = mybir.AluOpType.mult
    ADD = mybir.AluOpType.add

    with TileContext(nc) as tc:
        with tc.tile_pool(name="io", bufs=2) as io, \
             tc.tile_pool(name="mid", bufs=2) as mid, \
             tc.tile_pool(name="fix", bufs=1) as fix:
            sd = fix.tile([P, 2 * nchunk], _F32)
            sa = fix.tile([P, 2 * nchunk], _F32)
            sp = fix.tile([P, nchunk], _F32)
            scr_v = fix.tile([P, f_chunk], _F32)   # DVE garbage out
            scr_p = fix.tile([P, f_chunk], _F32)   # DVE garbage out (pos count)

            for i in range(nchunk):
                sl = bass.ts(i, f_chunk)
                rp_t = io.tile([P, f_chunk], _F32, tag="rp")
                nc.sync.dma_start(out=rp_t[:], in_=rp[:, sl])
                rt_t = io.tile([P, f_chunk], _F32, tag="rt")
                nc.sync.dma_start(out=rt_t[:], in_=rt[:, sl])
                ap_t = io.tile([P, f_chunk], _F32, tag="ap")
                nc.sync.dma_start(out=ap_t[:], in_=ap[:, sl])
                at_t = io.tile([P, f_chunk], _F32, tag="at")
                nc.sync.dma_start(out=at_t[:], in_=at[:, sl])

                dr = mid.tile([P, f_chunk], _F32, tag="dr")
                nc.vector.tensor_sub(dr[:], rp_t[:], rt_t[:])
                da = mid.tile([P, f_chunk], _F32, tag="da")
                nc.vector.tensor_sub(da[:], ap_t[:], at_t[:])

                mx = mid.tile([P, f_chunk], _F32, tag="mx")
                nc.vector.tensor_max(mx[:], rt_t[:], at_t[:])

                sqr = mid.tile([P, f_chunk], _F32, tag="sqr")
                nc.scalar.activation(sqr[:], dr[:], SQ, accum_out=sa[:, i : i + 1])
                sqa = mid.tile([P, f_chunk], _F32, tag="sqa")
                nc.scalar.activation(
                    sqa[:], da[:], SQ, accum_out=sa[:, nchunk + i : nchunk + i + 1]
                )

                # n_pos partial: sum of (mx > 0.5)
                nc.vector.tensor_scalar(
                    scr_p[:], mx[:], 0.5, None, IS_GT, ADD, accum_out=sp[:, i : i + 1]
                )
                # masked sums: sum of (mx > 0.5) * sq
                nc.vector.scalar_tensor_tensor(
                    scr_v[:], mx[:], 0.5, sqr[:],
                    op0=IS_GT, op1=MULT, accum_out=sd[:, i : i + 1],
                )
                nc.vector.scalar_tensor_tensor(
                    scr_v[:], mx[:], 0.5, sqa[:],
                    op0=IS_GT, op1=MULT,
                    accum_out=sd[:, nchunk + i : nchunk + i + 1],
                )

            nc.sync.dma_start(out=sd_out[:], in_=sd[:])
            nc.sync.dma_start(out=sa_out[:], in_=sa[:])
            nc.sync.dma_start(out=sp_out[:], in_=sp[:])
    nc.compile()
    return nc


_NC_CACHE: dict = {}


def _get_nc() -> bass.Bass:
    if "nc" not in _NC_CACHE:
        _NC_CACHE["nc"] = build_nc()
    return _NC_CACHE["nc"]


def _shard(x: np.ndarray, c: int) -> np.ndarray:
    per_b = B // N_CORES
    return np.ascontiguousarray(x.reshape(B, H * W)[c * per_b : (c + 1) * per_b]).reshape(
        P, F_TOT
    )


def _host_fallback_topk(region_pred, affinity_pred, region_target, affinity_target,
                        n_pos, n_neg):
    """Exact OHEM (reference semantics) on host — unreachable for uniform data."""
    rlm = (region_pred.astype(np.float64) - region_target.astype(np.float64)) ** 2
    alm = (affinity_pred.astype(np.float64) - affinity_target.astype(np.float64)) ** 2
    pos = (region_target > 0.5) | (affinity_target > 0.5)
    neg = ~pos
    comb = ((rlm + alm) * neg).reshape(-1)
    idx = np.argsort(-comb, kind="stable")[:n_neg]
    neg_r = rlm.reshape(-1)[idx].mean()
    neg_a = alm.reshape(-1)[idx].mean()
    pos_r = (rlm * pos).sum() / n_pos
    pos_a = (alm * pos).sum() / n_pos
    return pos_r + neg_r, pos_a + neg_a


def kernel(region_pred, affinity_pred, region_target, affinity_target):
    region_pred = np.asarray(region_pred, dtype=np.float32)
    affinity_pred = np.asarray(affinity_pred, dtype=np.float32)
    region_target = np.asarray(region_target, dtype=np.float32)
    affinity_target = np.asarray(affinity_target, dtype=np.float32)

    nc = _get_nc()
    in_maps = [
        {
            "region_pred": _shard(region_pred, c),
            "affinity_pred": _shard(affinity_pred, c),
            "region_target": _shard(region_target, c),
            "affinity_target": _shard(affinity_target, c),
        }
        for c in range(N_CORES)
    ]
    res = run_bass_kernel_spmd(nc, in_maps, list(range(N_CORES))).results

    nchunk = F_TOT // F_CHUNK
    S_pos_r = S_pos_a = S_tot_r = S_tot_a = n_pos_f = 0.0
    for c in range(N_CORES):
        sd = res[c]["stats_dve"].astype(np.float64)
        sa = res[c]["stats_act"].astype(np.float64)
        sp = res[c]["stats_pool"].astype(np.float64)
        S_pos_r += sd[:, :nchunk].sum()
        S_pos_a += sd[:, nchunk:].sum()
        S_tot_r += sa[:, :nchunk].sum()
        S_tot_a += sa[:, nchunk:].sum()
        n_pos_f += sp.sum()

    n_pos = int(round(n_pos_f))
    n_neg_tot = N_TOTAL - n_pos

    if n_pos == 0:
        region_loss = S_tot_r / N_TOTAL
        affinity_loss = S_tot_a / N_TOTAL
    else:
        pos_r = S_pos_r / n_pos
        pos_a = S_pos_a / n_pos
        n_neg = min(n_neg_tot, int(n_pos * NEG_RATIO))
        if n_neg == 0:
            region_loss, affinity_loss = pos_r, pos_a
        elif n_neg == n_neg_tot:
            region_loss = pos_r + (S_tot_r - S_pos_r) / n_neg
            affinity_loss = pos_a + (S_tot_a - S_pos_a) / n_neg
        else:
            region_loss, affinity_loss = _host_fallback_topk(
                region_pred, affinity_pred, region_target, affinity_target,
                n_pos, n_neg,
            )

    total = np.float32(region_loss + affinity_loss)
    return (total, np.float32(region_loss), np.float32(affinity_loss))
